# revision 1
# baseline (speedup 1.0000x reference)
"""Trainium2 Bass kernel for multi-level bilinear grid interpolation
(embedding_lookup, nn_COOLCHIC_INTERP_ENC).

Strategy:
  - 8 NeuronCores, data-parallel over query points, sharded spatially:
    points are bucketed by latitude into 256 equal-angle bands
    (8 ranks x 4 sequential passes x 8 gpsimd cores per rank). Band
    tables (the few grid rows a band can touch at each pyramid level,
    with the lat+1-row clip baked in) are replicated per-partition in
    SBUF so GPSIMD ap_gather (per-core shared index stream) can do the
    data-dependent lookup.
  - Levels 0-1 use d=2 vertical-pair tables (2 gather indices per
    point: columns w and w+1); levels 2-7 use d=4 2x2-quad tables with
    the column clip also baked in (1 gather index per point).
  - Per point & level: DVE computes the floor/clip/frac index math
    with fp32 ops chosen to be bit-identical to the jax reference
    (exact power-of-2 scales, magic-number floor), emits int16 gather
    indices, ap_gather fetches from SBUF tables, results bounce
    through DRAM to undo the 16-wide index interleave, DVE does the
    bilinear lerp mirroring the reference expression order. Output is
    bit-exact vs the fp32 jax reference.
"""

import sys

sys.path.insert(0, "/opt/trn_rl_repo")

import numpy as np

from concourse import bacc, bass, mybir
import concourse.tile as tile

# ---------------------------------------------------------------- constants
H_GRID, W_GRID, LEVEL, RES = 721, 1440, 8, 0.25
N_RANKS = 8
N_PASSES = 4
N_Q7 = 8
BANDS = N_RANKS * N_PASSES * N_Q7  # 256
BAND_DEG = 180.0 / BANDS  # 0.703125
MAGIC = np.float32(2.0**23)

WL = [1440, 721, 361, 181, 91, 46, 24, 13]  # used grid width per level
CAP = [5, 4, 3, 3, 3, 3, 3, 3]  # table rows per band per level
ENT = [CAP[l] * WL[l] for l in range(LEVEL)]
# levels 0-1: d=2 vertical-pair tables (2 gather idx/point);
# levels 2-7: d=4 2x2-quad tables (1 gather idx/point, col clip baked in)
D2L = 2  # levels using d=2
BASE2 = [sum(ENT[:l]) for l in range(D2L)]
TE2 = sum(ENT[:D2L])  # 10084 entries x2 f32
BASE4 = [sum(ENT[D2L:l]) for l in range(D2L, LEVEL)]
TE4 = sum(ENT[D2L:])  # 2148 entries x4 f32
NMETA = 3 * LEVEL  # base', lo, hi per level

F32 = mybir.dt.float32
I16 = mybir.dt.int16


def _res(l):
    return RES * (2.0**l)


def _rs(b, l):
    """First table row (global grid row) for band b at level l."""
    return int(np.floor(b * BAND_DEG / _res(l)))


# ---------------------------------------------------------------- device kernel
def build_kernel(c_band, f, ni):
    """Build the per-rank SPMD Bass program.

    c_band: padded points per band (= 16 * 2 * f ... c_band = 16*f*n_batch)
    f: free-dim columns per batch (points per partition per batch)
    ni: indices per ap_gather call (per core)
    """
    assert c_band % (16 * f) == 0
    n_batch = c_band // (16 * f)
    stream = 2 * f * 16  # gather indices per core per (batch, level)
    assert stream % ni == 0
    nsub = stream // ni
    assert ni % 16 == 0 and ni % 4 == 0

    nc = bacc.Bacc(None, target_bir_lowering=False)
    jj = c_band // 16
    xs_t = nc.declare_dram_parameter("xs", [N_PASSES, N_Q7, 16, 2, jj], F32, False)
    tab_t = nc.declare_dram_parameter("tables", [N_PASSES, N_Q7, TE2, 2], F32, False)
    tab4_t = nc.declare_dram_parameter("tables4", [N_PASSES, N_Q7, TE4, 4], F32, False)
    meta_t = nc.declare_dram_parameter("meta", [N_PASSES, 128, NMETA], F32, False)
    out_t = nc.declare_dram_parameter(
        "out", [N_PASSES, N_Q7, 16, LEVEL, jj], F32, True)

    from contextlib import ExitStack

    with tile.TileContext(nc) as tc, ExitStack() as es:
        sb = es.enter_context(tc.tile_pool(name="sb", bufs=2))
        sc = es.enter_context(tc.tile_pool(name="sc", bufs=1))
        sb1 = es.enter_context(tc.tile_pool(name="sb1", bufs=1))
        sf = es.enter_context(tc.tile_pool(name="sf", bufs=2))
        sd = es.enter_context(tc.tile_pool(name="sd", bufs=2))
        dr = es.enter_context(tc.tile_pool(name="dr", bufs=2, space="DRAM"))

        for p in range(N_PASSES):
            tabs = sb1.tile([128, TE2, 2], F32, tag="tabs")
            tabs4 = sb1.tile([128, TE4, 4], F32, tag="tabs4")
            for q in range(16):
                nc.sync.dma_start(out=tabs[q::16], in_=tab_t[p])
                nc.sync.dma_start(out=tabs4[q::16], in_=tab4_t[p])
            meta = sb1.tile([128, NMETA], F32, tag="meta")
            nc.sync.dma_start(out=meta[:], in_=meta_t[p])

            xv = xs_t[p].rearrange("k q c j -> (k q) c j")  # [128, 2, c/16]
            ov = out_t[p].rearrange("k q l j -> (k q) l j")

            for bi in range(n_batch):
                jsl = slice(bi * f, (bi + 1) * f)
                lat = sc.tile([128, f], F32, tag="lat")
                lon = sc.tile([128, f], F32, tag="lon")
                nc.sync.dma_start(out=lat[:], in_=xv[:, 0, jsl])
                nc.sync.dma_start(out=lon[:], in_=xv[:, 1, jsl])
                t90 = sb.tile([128, f], F32, tag="t90")
                # t90 = 90 - lat  (exactly as reference computes it)
                nc.vector.tensor_scalar(
                    out=t90[:], in0=lat[:], scalar1=-1.0, scalar2=90.0,
                    op0=mybir.AluOpType.mult, op1=mybir.AluOpType.add)

                for l in range(LEVEL):
                    invr = 1.0 / _res(l)  # power of two -> exact
                    w_l = WL[l]
                    a = sc.tile([128, f], F32, tag="a")
                    nc.vector.tensor_scalar_mul(out=a[:], in0=t90[:], scalar1=invr)
                    o = sc.tile([128, f], F32, tag="o")
                    nc.vector.tensor_scalar_mul(out=o[:], in0=lon[:], scalar1=invr)

                    # floor via round-to-nearest magic + fixup (exact for 0<=x<2^22)
                    def ffloor(x, tag):
                        r = sc.tile([128, f], F32, tag=tag + "r")
                        nc.vector.tensor_scalar(
                            out=r[:], in0=x[:], scalar1=float(MAGIC),
                            scalar2=-float(MAGIC),
                            op0=mybir.AluOpType.add, op1=mybir.AluOpType.add)
                        g = sc.tile([128, f], F32, tag=tag + "g")
                        nc.vector.tensor_tensor(
                            out=g[:], in0=r[:], in1=x[:], op=mybir.AluOpType.is_gt)
                        nc.vector.tensor_tensor(
                            out=r[:], in0=r[:], in1=g[:], op=mybir.AluOpType.subtract)
                        return r

                    hf = ffloor(a, "hf")
                    # clamp to the band's valid local rows: [lo, hi]
                    nc.vector.tensor_scalar(
                        out=hf[:], in0=hf[:],
                        scalar1=meta[:, LEVEL + l : LEVEL + l + 1],      # lo
                        scalar2=meta[:, 2 * LEVEL + l : 2 * LEVEL + l + 1],  # hi
                        op0=mybir.AluOpType.max, op1=mybir.AluOpType.min)
                    fa = sf.tile([128, f], F32, tag="fa")
                    nc.vector.tensor_tensor(
                        out=fa[:], in0=a[:], in1=hf[:], op=mybir.AluOpType.subtract)

                    wf = ffloor(o, "wf")
                    nc.vector.tensor_scalar(
                        out=wf[:], in0=wf[:], scalar1=0.0, scalar2=float(w_l - 1),
                        op0=mybir.AluOpType.max, op1=mybir.AluOpType.min)
                    fb = sf.tile([128, f], F32, tag="fb")
                    nc.vector.tensor_tensor(
                        out=fb[:], in0=o[:], in1=wf[:], op=mybir.AluOpType.subtract)
                    # flat entry ids: q = hf*W + w + (base - rs*W)   [exact fp32]
                    hfw = sc.tile([128, f], F32, tag="hfw")
                    nc.vector.tensor_scalar(
                        out=hfw[:], in0=hf[:], scalar1=float(w_l),
                        scalar2=meta[:, l : l + 1],
                        op0=mybir.AluOpType.mult, op1=mybir.AluOpType.add)
                    qf = sc.tile([128, f], F32, tag="qf")
                    nc.vector.tensor_tensor(
                        out=qf[:], in0=hfw[:], in1=wf[:], op=mybir.AluOpType.add)

                    # gather, then undo the 16-wide stream interleave via a
                    # DRAM bounce. vfull per point j: [ff, cf, fc, cc]
                    vfull = sb.tile([128, 4 * f], F32, tag="vfull")
                    vv = vfull[:].rearrange("p (j z r) -> p j z r", z=2, r=2)

                    if l < D2L:
                        # two d=2 gathers per point: columns wf and wc
                        wc = sc.tile([128, f], F32, tag="wfg")
                        nc.vector.tensor_scalar(
                            out=wc[:], in0=wf[:], scalar1=1.0, scalar2=float(w_l - 1),
                            op0=mybir.AluOpType.add, op1=mybir.AluOpType.min)
                        idx = sb.tile([128, 2 * f], I16, tag="idx")
                        iv = idx[:].rearrange("p (j z) -> p j z", z=2)
                        nc.vector.tensor_copy(out=iv[:, :, 0], in_=qf[:])
                        nc.vector.tensor_tensor(
                            out=iv[:, :, 1], in0=hfw[:], in1=wc[:],
                            op=mybir.AluOpType.add)
                        bnc = dr.tile([N_Q7, nsub * ni, 2], F32, tag="bnc")
                        for s in range(nsub):
                            dst = sd.tile([128, ni, 2], F32, tag="dst")
                            nc.gpsimd.ap_gather(
                                dst[:], tabs[:],
                                idx[:, s * (ni // 16):(s + 1) * (ni // 16)],
                                channels=128, num_elems=TE2, d=2, num_idxs=ni)
                            nc.sync.dma_start(
                                out=bnc[:, s * ni : (s + 1) * ni], in_=dst[::16])
                        # dst col (2j+z)*16+q holds [r0, r1] -> vv[:, j, z, r]
                        bq = bnc[:].rearrange("k (i q) r -> k q i r", q=16)
                        for q in range(16):
                            nc.sync.dma_start(out=vfull[q::16], in_=bq[:, q])
                    else:
                        # one d=4 gather per point (quad with clips baked in)
                        ni4 = ni // 2
                        nsub4 = (f * 16) // ni4
                        idx = sb.tile([128, 2 * f], I16, tag="idx")
                        nc.vector.tensor_copy(out=idx[:, :f], in_=qf[:])
                        bnc = dr.tile([N_Q7, nsub * ni, 2], F32, tag="bnc")
                        b4 = bnc[:].rearrange("k i r -> k (i r)").rearrange(
                            "k (i c) -> k i c", c=4)  # [8, nsub4*ni4, 4]
                        for s in range(nsub4):
                            dst = sd.tile([128, ni4, 4], F32, tag="dst")
                            nc.gpsimd.ap_gather(
                                dst[:], tabs4[:],
                                idx[:, s * (ni4 // 16):(s + 1) * (ni4 // 16)],
                                channels=128, num_elems=TE4, d=4, num_idxs=ni4)
                            nc.sync.dma_start(
                                out=b4[:, s * ni4 : (s + 1) * ni4], in_=dst[::16])
                        # dst col j*16+q holds the 4-quad -> vfull[16k+q, 4j:4j+4]
                        bq = b4.rearrange("k (i q) c -> k q i c", q=16)
                        for q in range(16):
                            nc.sync.dma_start(out=vfull[q::16], in_=bq[:, q])

                    # bilinear lerp, matching reference expression order
                    vf = sc.tile([128, f], F32, tag="vf")
                    vc = sc.tile([128, f], F32, tag="vc")
                    res = sb.tile([128, f], F32, tag="res")

                    def lerp(outt, v0, v1, fr):
                        nc.vector.tensor_tensor(
                            out=outt[:], in0=v1, in1=v0, op=mybir.AluOpType.subtract)
                        nc.vector.tensor_tensor(
                            out=outt[:], in0=outt[:], in1=fr[:], op=mybir.AluOpType.mult)
                        nc.vector.tensor_tensor(
                            out=outt[:], in0=outt[:], in1=v0, op=mybir.AluOpType.add)

                    lerp(vf, vv[:, :, 0, 0], vv[:, :, 1, 0], fb)
                    lerp(vc, vv[:, :, 0, 1], vv[:, :, 1, 1], fb)
                    lerp(res, vf[:], vc[:], fa)
                    nc.sync.dma_start(out=ov[:, l, jsl], in_=res[:])

    nc.compile()
    return nc


# ---------------------------------------------------------------- host tables
def build_tables(emb):
    """emb: [LEVEL, 721, 1440] -> tables2 [BANDS, TE2, 2], tables4
    [BANDS, TE4, 4] (entry = [ff, cf, fc, cc]), meta [BANDS, NMETA]."""
    tables2 = np.zeros((BANDS, TE2, 2), np.float32)
    tables4 = np.zeros((BANDS, TE4, 4), np.float32)
    meta = np.zeros((BANDS, NMETA), np.float32)
    b = np.arange(BANDS)
    for l in range(LEVEL):
        w_l = WL[l]
        rs = np.floor(b * BAND_DEG / _res(l)).astype(np.int64)  # [BANDS]
        rows = np.minimum(rs[:, None] + np.arange(CAP[l])[None, :], H_GRID - 1)
        rows2 = np.minimum(rows + 1, H_GRID - 1)
        g0 = emb[l][rows][:, :, :w_l]  # [BANDS, CAP, W] row h
        g1 = emb[l][rows2][:, :, :w_l]  # row h+1 (clipped)
        if l < D2L:
            blk = np.stack([g0, g1], axis=-1).reshape(BANDS, ENT[l], 2)
            tables2[:, BASE2[l] : BASE2[l] + ENT[l]] = blk
            base = BASE2[l]
        else:
            cols2 = np.minimum(np.arange(w_l) + 1, w_l - 1)  # baked col clip
            blk = np.stack(
                [g0, g1, g0[:, :, cols2], g1[:, :, cols2]], axis=-1
            ).reshape(BANDS, ENT[l], 4)
            tables4[:, BASE4[l - D2L] : BASE4[l - D2L] + ENT[l]] = blk
            base = BASE4[l - D2L]
        meta[:, l] = (base - rs * w_l).astype(np.float32)  # base'
        meta[:, LEVEL + l] = rs.astype(np.float32)  # lo
        meta[:, 2 * LEVEL + l] = np.minimum(rs + CAP[l] - 2, H_GRID - 1).astype(
            np.float32)  # hi
    return tables2, tables4, meta


def shard_points(x, c_band):
    """Bucket points into BANDS latitude bands; returns padded xs
    [BANDS, c_band, 2], plus (order, counts) to invert."""
    lat64 = x[:, 0].astype(np.float64)
    b = np.clip(np.floor((90.0 - lat64) / BAND_DEG).astype(np.int64), 0, BANDS - 1)
    order = np.argsort(b, kind="stable")
    counts = np.bincount(b, minlength=BANDS)
    if counts.max() > c_band:
        raise ValueError(f"band overflow: {counts.max()} > {c_band}")
    xs = np.zeros((BANDS, c_band, 2), np.float32)
    centers = (90.0 - (np.arange(BANDS) + 0.5) * BAND_DEG).astype(np.float32)
    xs[:, :, 0] = centers[:, None]
    xsorted = x[order]
    off = 0
    for bb in range(BANDS):
        n = counts[bb]
        xs[bb, :n] = xsorted[off : off + n]
        off += n
    return xs, order, counts


def unshard_output(res_out, order, counts, n_points):
    """res_out: [BANDS, c_band, LEVEL] -> [n_points, LEVEL] in original order."""
    parts = [res_out[bb, : counts[bb]] for bb in range(BANDS)]
    sorted_out = np.concatenate(parts, axis=0)
    out = np.empty((n_points, LEVEL), np.float32)
    out[order] = sorted_out
    return out


# ---------------------------------------------------------------- entry point
_NC_CACHE = {}
LAST_RESULT = None

C_BAND_HW = 16384
F_HW = 256
NI_HW = 4096


def kernel(x, embeddings):
    global LAST_RESULT
    from concourse.bass_utils import run_bass_kernel_spmd

    x = np.ascontiguousarray(np.asarray(x), dtype=np.float32)
    emb = np.asarray(embeddings, dtype=np.float32)
    n = x.shape[0]

    tables2, tables4, meta = build_tables(emb)
    # pick a band capacity that fits the actual point distribution
    c_band, f_hw = C_BAND_HW, F_HW
    lat64 = x[:, 0].astype(np.float64)
    bmax = int(np.bincount(
        np.clip(np.floor((90.0 - lat64) / BAND_DEG).astype(np.int64), 0, BANDS - 1),
        minlength=BANDS).max())
    while bmax > c_band:
        c_band *= 2
    key = (c_band, f_hw, NI_HW)
    if key not in _NC_CACHE:
        _NC_CACHE[key] = build_kernel(*key)
    nc = _NC_CACHE[key]

    xs, order, counts = shard_points(x, c_band)

    # [BANDS,...] -> per rank [N_PASSES, N_Q7, ...]; band = 32r + 8p + k
    # partition-wrap each band: [c,2] -> [16, c//16, 2] (point i -> (i%16, i//16))
    C_BAND = c_band
    jj = C_BAND // 16
    xs_r = xs.reshape(N_RANKS, N_PASSES, N_Q7, jj, 16, 2).transpose(0, 1, 2, 4, 5, 3)
    tab_r = tables2.reshape(N_RANKS, N_PASSES, N_Q7, TE2, 2)
    tab4_r = tables4.reshape(N_RANKS, N_PASSES, N_Q7, TE4, 4)
    meta_r = np.broadcast_to(
        meta.reshape(N_RANKS, N_PASSES, N_Q7, 1, NMETA),
        (N_RANKS, N_PASSES, N_Q7, 16, NMETA),
    ).reshape(N_RANKS, N_PASSES, 128, NMETA)

    in_maps = [
        {
            "xs": np.ascontiguousarray(xs_r[r]),
            "tables": np.ascontiguousarray(tab_r[r]),
            "tables4": np.ascontiguousarray(tab4_r[r]),
            "meta": np.ascontiguousarray(meta_r[r]),
        }
        for r in range(N_RANKS)
    ]
    kres = run_bass_kernel_spmd(nc, in_maps, list(range(N_RANKS)))
    LAST_RESULT = kres
    results = kres.results
    res = np.stack([results[r]["out"] for r in range(N_RANKS)])  # [R,P,K,16,L,J]
    res = res.transpose(0, 1, 2, 5, 3, 4).reshape(BANDS, C_BAND, LEVEL)
    return unshard_output(res, order, counts, n)



# revision 9
# speedup vs baseline: 33.7367x; 33.7367x over previous
"""Trainium2 Bass kernel for multi-level bilinear grid interpolation
(embedding_lookup, nn_COOLCHIC_INTERP_ENC).

Strategy (v3 — per-cell coefficient table + dma_gather + run sharing):
  - For each level-0 grid cell and each pyramid level l, the bilinear
    interpolation restricted to that cell is
        out_l(u0, v0) = A + B*u0 + C*v0 + D*u0*v0
    in the point's level-0 fracs (u0, v0). Host precomputes
    [A0..7, B0..7, C0..7, D0..7] (32 fp16, scaled 2^13) per cell into a
    256B-strided DRAM table.
  - 8 NeuronCores, data-parallel over latitude: rank r owns 5 windows
    of 18 grid rows (90 rows each). Host sorts points by cell, groups
    same-cell runs into slots of R points, and lays slots out so one
    256B dma_gather descriptor serves R points (the coefficient operand
    is read through a stride-0 broadcast AP).
  - Device per tile: Act/DVE compute exact floors and fracs, a gpsimd
    dma_gather fetches one coeff entry per slot, the 8-level Horner
    lerp runs on DVE in fp16 (level dim packed -> 2x mode), final
    unscale on Act, contiguous DMA writes [*, 8] f32 per point.
"""

import sys

sys.path.insert(0, "/opt/trn_rl_repo")

import numpy as np

from concourse import bacc, bass, mybir
import concourse.tile as tile

# ---------------------------------------------------------------- constants
H_GRID, W_GRID, LEVEL, RES = 721, 1440, 8, 0.25
NCELLS = H_GRID * W_GRID
ENTRY = 4 * LEVEL  # 32 fp16 coeffs per cell
SLOT_F32 = 64  # 256B table slot (fp16 coeffs in first 16 f32 words)
CSCALE = 2.0**13
N_RANKS = 8
ROWS_WIN = 18
NWIN = 40  # windows 0..38: 18 rows; window 39: 19 rows
WPC = NWIN // N_RANKS  # windows per core
WIN_ENT = (ROWS_WIN + 1) * W_GRID  # table entries per window slice (max)
T_TILES = 8  # tiles per window
MAGIC = np.float32(2.0**23)

F32 = mybir.dt.float32
F16 = mybir.dt.float16
I16 = mybir.dt.int16

R_HW = 4  # points per slot (per 256B descriptor)

# kept so external harnesses poking legacy knobs don't crash
C_BAND_HW, F_HW, NI_HW = 0, 0, 0


# ---------------------------------------------------------------- device kernel
def build_kernel(s_slots, r):
    """Per-rank SPMD program. s_slots: coeff slots per partition per
    tile; r: points per slot. J = s_slots*r points per partition."""
    j = s_slots * r
    num_idxs = 128 * s_slots

    nc = bacc.Bacc(None, target_bir_lowering=False)
    xs_t = nc.declare_dram_parameter("xs", [WPC, T_TILES, 128, j, 2], F32, False)
    idx_t = nc.declare_dram_parameter(
        "idx", [WPC, T_TILES, 128, 8 * s_slots], I16, False)
    tab_t = nc.declare_dram_parameter("table", [WPC, WIN_ENT, SLOT_F32], F32, False)
    out_t = nc.declare_dram_parameter("out", [WPC, T_TILES, 128, j, LEVEL], F32, True)

    AF = mybir.ActivationFunctionType
    from contextlib import ExitStack

    with tile.TileContext(nc) as tc, ExitStack() as es:
        sb = es.enter_context(tc.tile_pool(name="sb", bufs=2))

        for w in range(WPC):
            for t in range(T_TILES):
                xt = sb.tile([128, j, 2], F32, tag="xt")
                nc.sync.dma_start(out=xt[:], in_=xs_t[w, t])
                idxt = sb.tile([128, 8 * s_slots], I16, tag="idxt")
                nc.sync.dma_start(out=idxt[:], in_=idx_t[w, t])
                lat = xt[:, :, 0]
                lon = xt[:, :, 1]

                # a0 = (90 - lat)*4 ; o0 = lon*4  (exact vs reference)
                a0 = sb.tile([128, j], F32, tag="a0")
                nc.scalar.activation(out=a0[:], in_=lat, func=AF.Copy,
                                     scale=-4.0, bias=360.0)
                o0 = sb.tile([128, j], F32, tag="o0")
                nc.scalar.activation(out=o0[:], in_=lon, func=AF.Copy, scale=4.0)

                # exact floor: rr = (x + 2^23) - 2^23 ; rr -= (rr > x)
                def ffloor(x, tag):
                    rr = sb.tile([128, j], F32, tag=tag + "r")
                    nc.vector.tensor_scalar(
                        out=rr[:], in0=x[:], scalar1=float(MAGIC),
                        scalar2=-float(MAGIC),
                        op0=mybir.AluOpType.add, op1=mybir.AluOpType.add)
                    g = sb.tile([128, j], F32, tag=tag + "g")
                    nc.vector.tensor_tensor(
                        out=g[:], in0=rr[:], in1=x[:], op=mybir.AluOpType.is_gt)
                    nc.vector.tensor_tensor(
                        out=rr[:], in0=rr[:], in1=g[:], op=mybir.AluOpType.subtract)
                    return rr

                h0 = ffloor(a0, "h")
                w0 = ffloor(o0, "w")
                # fracs in place: u0 = a0 - h0 ; v0 = o0 - w0
                nc.vector.tensor_tensor(
                    out=a0[:], in0=a0[:], in1=h0[:], op=mybir.AluOpType.subtract)
                nc.vector.tensor_tensor(
                    out=o0[:], in0=o0[:], in1=w0[:], op=mybir.AluOpType.subtract)

                u16 = sb.tile([128, j], F16, tag="u16")
                nc.scalar.activation(out=u16[:], in_=a0[:], func=AF.Copy)
                v16 = sb.tile([128, j], F16, tag="v16")
                nc.scalar.activation(out=v16[:], in_=o0[:], func=AF.Copy)
                urep = sb.tile([128, j, LEVEL], F16, tag="urep")
                nc.scalar.activation(
                    out=urep[:], func=AF.Copy,
                    in_=u16[:].rearrange("p (j o) -> p j o", o=1)
                    .to_broadcast([128, j, LEVEL]))
                vrep = sb.tile([128, j, LEVEL], F16, tag="vrep")
                nc.scalar.activation(
                    out=vrep[:], func=AF.Copy,
                    in_=v16[:].rearrange("p (j o) -> p j o", o=1)
                    .to_broadcast([128, j, LEVEL]))

                # one 256B descriptor per slot
                g = sb.tile([128, s_slots, SLOT_F32], F32, tag="g")
                nc.gpsimd.dma_gather(
                    out_ap=g[:], in_ap=tab_t[w], idxs_ap=idxt[:],
                    num_idxs=num_idxs, num_idxs_reg=num_idxs,
                    elem_size=SLOT_F32)
                gh = g[:].bitcast(F16)  # [128, s_slots, 128]

                def cblk(k):  # [128, s, r(bcast), 8] fp16 coeff view
                    return (gh[:, :, k * LEVEL:(k + 1) * LEVEL]
                            .rearrange("p (s o) e -> p s o e", o=1)
                            .to_broadcast([128, s_slots, r, LEVEL]))

                cA, cB, cC, cD = cblk(0), cblk(1), cblk(2), cblk(3)
                uv = urep[:].rearrange("p (s r) e -> p s r e", r=r)
                vv = vrep[:].rearrange("p (s r) e -> p s r e", r=r)

                # Horner: out = (A + B*u) + (C + D*u)*v  [fp16, 2x mode]
                t2 = sb.tile([128, j, LEVEL], F16, tag="t2")
                t2v = t2[:].rearrange("p (s r) e -> p s r e", r=r)
                nc.vector.tensor_tensor(out=t2v, in0=cB, in1=uv,
                                        op=mybir.AluOpType.mult)
                nc.vector.tensor_tensor(out=t2v, in0=t2v, in1=cA,
                                        op=mybir.AluOpType.add)
                t1 = sb.tile([128, j, LEVEL], F16, tag="t1")
                t1v = t1[:].rearrange("p (s r) e -> p s r e", r=r)
                nc.vector.tensor_tensor(out=t1v, in0=cD, in1=uv,
                                        op=mybir.AluOpType.mult)
                nc.vector.tensor_tensor(out=t1v, in0=t1v, in1=cC,
                                        op=mybir.AluOpType.add)
                nc.vector.tensor_tensor(out=t1v, in0=t1v, in1=vv,
                                        op=mybir.AluOpType.mult)

                res = sb.tile([128, j, LEVEL], F32, tag="res")
                nc.vector.tensor_tensor(out=res[:], in0=t1[:], in1=t2[:],
                                        op=mybir.AluOpType.add)
                nc.scalar.activation(out=res[:], in_=res[:], func=AF.Copy,
                                     scale=float(1.0 / CSCALE))
                nc.sync.dma_start(out=out_t[w, t], in_=res[:])

    nc.compile()
    return nc


# ---------------------------------------------------------------- host tables
def build_table(emb):
    """emb: [LEVEL, 721, 1440] f32 -> [NCELLS, 64] f32 table; first 16
    f32 words of each slot hold the 32 fp16 coeffs [A0..7,B..,C..,D..],
    scaled by 2^13."""
    co = np.empty((H_GRID, W_GRID, ENTRY), np.float16)
    h0 = np.arange(H_GRID)
    w0 = np.arange(W_GRID)
    for l in range(LEVEL):
        s = 2.0**-l
        hl = h0 >> l
        wl = w0 >> l
        al = ((h0 & ((1 << l) - 1)) * s)[:, None]
        ga = ((w0 & ((1 << l) - 1)) * s)[None, :]
        r0 = hl
        r1 = np.minimum(hl + 1, H_GRID - 1)
        c0 = wl
        c1 = np.minimum(wl + 1, W_GRID - 1)
        g = emb[l].astype(np.float64)
        q00 = g[r0][:, c0]
        q01 = g[r0][:, c1]
        q10 = g[r1][:, c0]
        q11 = g[r1][:, c1]
        dv = q01 - q00
        du = q10 - q00
        dd = q11 - q10 - q01 + q00
        co[:, :, 0 * LEVEL + l] = ((q00 + dv * ga + du * al + dd * (al * ga))
                                   * CSCALE).astype(np.float16)
        co[:, :, 1 * LEVEL + l] = ((du * s + dd * (s * ga)) * CSCALE).astype(
            np.float16)
        co[:, :, 2 * LEVEL + l] = ((dv * s + dd * (al * s)) * CSCALE).astype(
            np.float16)
        co[:, :, 3 * LEVEL + l] = (dd * (s * s) * CSCALE).astype(np.float16)
    tab = np.zeros((NCELLS, SLOT_F32), np.float32)
    tab[:, :ENTRY // 2] = co.reshape(NCELLS, ENTRY).view(np.float32)
    return tab


# ---------------------------------------------------------------- entry point
_NC_CACHE = {}
LAST_RESULT = None


def kernel(x, embeddings):
    global LAST_RESULT
    from concourse.bass_utils import run_bass_kernel_spmd

    x = np.ascontiguousarray(np.asarray(x), dtype=np.float32)
    emb = np.asarray(embeddings, dtype=np.float32)
    n = x.shape[0]
    r = R_HW

    # exact f32 index math (identical to device)
    lat = x[:, 0]
    lon = x[:, 1]
    a0 = (np.float32(90.0) - lat) * np.float32(4.0)
    o0 = lon * np.float32(4.0)
    h0 = np.floor(a0).astype(np.int64)
    w0 = np.floor(o0).astype(np.int64)
    cell = h0 * W_GRID + w0
    order = np.argsort(cell, kind="stable")
    cells_s = cell[order]
    win_s = np.minimum(cells_s // (W_GRID * ROWS_WIN), NWIN - 1)
    wbound = np.searchsorted(win_s, np.arange(NWIN + 1))

    # per-window run/slot assembly
    slot_entry_w = []
    ppos_w = []  # padded position of each real (sorted) point
    nslots_w = np.zeros(NWIN, np.int64)
    for w in range(NWIN):
        lo, hi = wbound[w], wbound[w + 1]
        cw = cells_s[lo:hi]
        base = w * ROWS_WIN * W_GRID
        if hi == lo:
            slot_entry_w.append(np.zeros(0, np.int64))
            ppos_w.append(np.zeros(0, np.int64))
            continue
        chg = np.r_[True, cw[1:] != cw[:-1]]
        starts = np.flatnonzero(chg)
        runcnt = np.diff(np.r_[starts, cw.size])
        runvals = cw[starts]
        spr = -(-runcnt // r)  # slots per run
        nslots_w[w] = spr.sum()
        slot_entry_w.append(np.repeat(runvals - base, spr))
        pcum = np.r_[0, np.cumsum(spr * r)][:-1]
        within = np.arange(cw.size) - np.repeat(starts, runcnt)
        ppos_w.append(np.repeat(pcum, runcnt) + within)

    s_slots = max(1, int(-(-nslots_w.max() // (128 * T_TILES))))
    j = s_slots * r
    cap = 128 * s_slots * T_TILES
    assert nslots_w.max() <= cap

    key = (s_slots, r)
    if key not in _NC_CACHE:
        _NC_CACHE[key] = build_kernel(s_slots, r)
    nc = _NC_CACHE[key]

    tab = build_table(emb)

    xs = np.empty((NWIN, T_TILES, 128, j, 2), np.float32)
    xs[..., 0] = 90.0
    xs[..., 1] = 0.0
    idxa = np.zeros((NWIN, T_TILES, 128, 8 * s_slots), np.int16)
    dst_w = []  # flat output position per real (sorted) point, per window
    for w in range(NWIN):
        se = slot_entry_w[w]
        if se.size:
            s_arr = np.arange(se.size)
            t_a = s_arr // (128 * s_slots)
            s2 = s_arr % (128 * s_slots)
            i_a = s2 // 128
            p_a = s2 % 128
            idxa[w, t_a, 16 + p_a % 16, p_a // 16 + 8 * i_a] = se.astype(np.int16)
        # real point padded positions -> (t, p, j)
        q = ppos_w[w]
        s_arr = q // r
        r_a = q % r
        t_a = s_arr // (128 * s_slots)
        s2 = s_arr % (128 * s_slots)
        i_a = s2 // 128
        p_a = s2 % 128
        lo = wbound[w]
        pts = order[lo:wbound[w + 1]]
        xs[w, t_a, p_a, i_a * r + r_a] = x[pts]
        dst_w.append(((t_a * 128 + p_a) * j + i_a * r + r_a))

    # per-rank table slices
    in_maps = []
    for rank in range(N_RANKS):
        tabr = np.zeros((WPC, WIN_ENT, SLOT_F32), np.float32)
        for w in range(WPC):
            gw = rank * WPC + w
            base = gw * ROWS_WIN * W_GRID
            nent = NCELLS - base if gw == NWIN - 1 else ROWS_WIN * W_GRID
            # runs may reference cells one row past the window (never
            # happens: cells are binned by their own row) — slice exact.
            tabr[w, :nent] = tab[base:base + nent]
        in_maps.append({
            "xs": np.ascontiguousarray(xs[rank * WPC:(rank + 1) * WPC]),
            "idx": np.ascontiguousarray(idxa[rank * WPC:(rank + 1) * WPC]),
            "table": tabr,
        })

    kres = run_bass_kernel_spmd(nc, in_maps, list(range(N_RANKS)))
    LAST_RESULT = kres
    results = kres.results

    out = np.empty((n, LEVEL), np.float32)
    for rank in range(N_RANKS):
        res = results[rank]["out"]  # [WPC, T, 128, j, LEVEL]
        for w in range(WPC):
            gw = rank * WPC + w
            lo, hi = wbound[gw], wbound[gw + 1]
            if hi > lo:
                flat = res[w].reshape(T_TILES * 128 * j, LEVEL)
                out[order[lo:hi]] = flat[dst_w[gw]]
    return out
